# revision 30
# baseline (speedup 1.0000x reference)
"""DotGatConv (DGL) on 8 Trainium2 NeuronCores — v5.

Strategy (vertex-cut / dst-partitioned, host-side edge materialization):
  - Nodes are split into 8 contiguous blocks of 12500 (padded to 12544),
    degree-sorted within each core and packed 128 per chunk
    (node-per-partition); each chunk's edge slots are padded to the chunk
    max degree S_c (shared schedule across cores so the SPMD program is
    identical on every core).
  - Sharding ships, per core, the transposed source features of every edge
    slot (feat[src] pre-arranged by the host = the "all-to-all gather of
    remote source features" done at sharding time).  The device then
    computes h_src per edge slot on the PE (feat_src @ W.T, bf16 in / fp32
    PSUM), so all model FLOPs stay on device and all edge bytes stream
    through HBM as fat sequential DMA loads instead of 205k random 512B
    indirect-DMA descriptors (which bottleneck on the ~1us/instruction
    software-DGE descriptor generation).
  - Per chunk: h_src slots PE->PSUM, evacuated to SBUF bf16 (alternating
    Scalar/Vector engines); per-edge dot scores via one fused mul+running-
    sum DVE scan (fp32 state) with group sums as boundary differences
    (extraction/subtraction on the otherwise idle GpSimd); softmax with
    pad-slot correction; sa expanded over the feature dim on ACT; applied
    in-place with a 2x-rate all-bf16 DVE multiply; aggregation over edge
    slots on the PE (identity-stationary matmuls accumulating into PSUM).
  - No AllGather and no h table: the only collective-ish step is the local
    fc for the chunk's own (dst) rows.
"""

import numpy as np

IN_SIZE = 256
OUT_SIZE = 32
NUM_HEADS = 8
HD = NUM_HEADS * OUT_SIZE  # 256
N_CORES = 8
CHUNK = 128
KT = IN_SIZE // 128  # contraction k-tiles (2)

# Real-problem dimensions (overridable for scaled-down sim tests).
N_NODES = 100000
NODES_PER_CORE = N_NODES // N_CORES  # 12500
N_CHUNKS = (NODES_PER_CORE + CHUNK - 1) // CHUNK  # 98
NODES_PAD = N_CHUNKS * CHUNK  # 12544

EVAC_GROUP = 4  # slots per PSUM->SBUF evacuation copy

_CACHE = {}
TRACE = False  # set by test harness to capture an NTFF profile
LAST_RESULT = None


def _set_dims(n_nodes):
    """Recompute derived dims (used by sim tests with small graphs)."""
    global N_NODES, NODES_PER_CORE, N_CHUNKS, NODES_PAD
    N_NODES = n_nodes
    NODES_PER_CORE = N_NODES // N_CORES
    N_CHUNKS = (NODES_PER_CORE + CHUNK - 1) // CHUNK
    NODES_PAD = N_CHUNKS * CHUNK


# --------------------------------------------------------------------------- #
# Custom DVE op: out = running_sum(in0 * in1) along the free dim (fp32 state).
# --------------------------------------------------------------------------- #
def _install_custom_op():
    import concourse.dve_ops as dve_ops
    from concourse.dve_spec import Scan, Spec, Src0, Src1, AluOp, lower
    from concourse.dve_uop import DveOpSpec

    if "GAT_MUL_SCAN" in dve_ops.CUSTOM_DVE_SPECS:
        return

    def _ref_mul_scan(in0, in1, s0, s1, imm2):
        p = in0.shape[0]
        a = np.asarray(in0, np.float32).reshape(p, -1)
        b = np.asarray(in1, np.float32)
        if b.size != a.size:
            b = np.broadcast_to(b.reshape(p, -1), a.shape)
        else:
            b = b.reshape(p, -1)
        prod = a * b
        return np.cumsum(prod, axis=1, dtype=np.float32).astype(np.float32)

    spec = Spec(body=Scan(AluOp.ADD, Src0 * Src1), reference=_ref_mul_scan)
    shas = {}
    for ver in ("v3", "v4"):
        uops = lower(spec, ver=ver)
        shas[ver] = DveOpSpec(
            name="GAT_MUL_SCAN", opcode=0, uops=uops, rd1_en=True
        ).sha(ver)
    op = dve_ops.DveOp("GAT_MUL_SCAN", spec, subdim=False, uops_sha=shas)
    dve_ops.OPS.append(op)
    dve_ops.CUSTOM_DVE_SPECS[op.name] = op.spec
    dve_ops._SUB_OPCODE_FOR_NAME[op.name] = dve_ops._CUSTOM_DVE_ROW_BASE + len(dve_ops.OPS) - 1


def _get_scan_op():
    import concourse.dve_ops as dve_ops

    _install_custom_op()
    for op in dve_ops.OPS:
        if op.name == "GAT_MUL_SCAN":
            return op
    raise RuntimeError("GAT_MUL_SCAN not installed")


# --------------------------------------------------------------------------- #
# Host-side sharding: group edges by dst core / degree-sorted node chunks.
# --------------------------------------------------------------------------- #
def build_shards(feat, W, src, dst):
    import ml_dtypes

    bf16 = ml_dtypes.bfloat16
    feat = np.ascontiguousarray(np.asarray(feat, dtype=np.float32))
    W = np.ascontiguousarray(np.asarray(W, dtype=np.float32))
    src = np.asarray(src).astype(np.int64)
    dst = np.asarray(dst).astype(np.int64)
    E = src.shape[0]

    dst_core = dst // NODES_PER_CORE
    dst_local = dst - dst_core * NODES_PER_CORE

    deg = np.bincount(dst, minlength=N_NODES)  # [N]

    # Degree-sort nodes within each core; identical rank structure per core.
    perms = np.empty((N_CORES, NODES_PER_CORE), dtype=np.int64)
    degs_sorted = np.empty((N_CORES, NODES_PER_CORE), dtype=np.int64)
    for c in range(N_CORES):
        d = deg[c * NODES_PER_CORE : (c + 1) * NODES_PER_CORE]
        p = np.argsort(d, kind="stable")
        perms[c] = p
        degs_sorted[c] = d[p]

    # Shared chunk schedule: S_c = max degree among rank-slice across cores.
    S = np.zeros(N_CHUNKS, dtype=np.int64)
    for c in range(N_CHUNKS):
        lo, hi = c * CHUNK, min((c + 1) * CHUNK, NODES_PER_CORE)
        S[c] = int(degs_sorted[:, lo:hi].max()) if hi > lo else 0
    S = np.maximum(S, 1)  # avoid zero-width chunks
    S_tot = int(S.sum())
    chunk_off = np.concatenate([[0], np.cumsum(S)])[:-1]

    # rank of each node within its core (inverse permutation)
    rank_of_local = np.empty((N_CORES, NODES_PER_CORE), dtype=np.int64)
    for c in range(N_CORES):
        rank_of_local[c, perms[c]] = np.arange(NODES_PER_CORE)

    # per-edge: core, rank, slot-within-node
    e_rank = rank_of_local[dst_core, dst_local]  # [E]
    order = np.lexsort((np.arange(E), e_rank, dst_core))  # stable by (core, rank)
    sorted_key = dst_core[order] * NODES_PER_CORE + e_rank[order]
    first = np.concatenate([[True], sorted_key[1:] != sorted_key[:-1]])
    grp_start = np.where(first)[0]
    grp_id = np.cumsum(first) - 1
    slot_sorted = np.arange(E) - grp_start[grp_id]
    slot = np.empty(E, dtype=np.int64)
    slot[order] = slot_sorted

    e_chunk = e_rank // CHUNK
    e_part = e_rank % CHUNK
    col = chunk_off[e_chunk] + slot  # column in [128, S_tot]

    # src node id per (core, partition, slot column); -1 marks pad slots
    src_of = np.full((N_CORES, CHUNK, S_tot), -1, dtype=np.int64)
    src_of[dst_core, e_part, col] = src

    # -(number of pad slots) per (partition, chunk), per core.  Pad slots
    # have zero source features -> score 0 -> ex = exp(0) = 1 exactly,
    # corrected by subtracting n_pad from the softmax denominator.
    npad = np.zeros((N_CORES, CHUNK, N_CHUNKS), dtype=np.float32)
    for c in range(N_CORES):
        dd = np.zeros(NODES_PAD, dtype=np.int64)
        dd[:NODES_PER_CORE] = degs_sorted[c]
        npad[c] = -(S[None, :] - dd.reshape(N_CHUNKS, CHUNK).T).astype(np.float32)

    featbf = feat.astype(bf16)
    featbf_pad = np.concatenate(
        [featbf, np.zeros((1, IN_SIZE), dtype=bf16)], axis=0
    )  # row N_NODES = zeros for pad slots

    # featT padded per core (dst-side fc), columns in degree-rank order
    featT = np.zeros((N_CORES, IN_SIZE, NODES_PAD), dtype=bf16)
    for c in range(N_CORES):
        featT[c, :, :NODES_PER_CORE] = featbf[c * NODES_PER_CORE + perms[c]].T
    WT = np.ascontiguousarray(W.T).astype(bf16)  # [IN, HD]
    ident = np.eye(CHUNK, dtype=bf16)

    # Pre-arranged per-edge source features, transposed for the PE:
    # per chunk c a [128, KT*S_c*128] block whose column (t*S_c + s)*128 + p
    # holds feat[src(c, p, s), t*128 + i] at partition i.
    fsrcT = np.empty((N_CORES, 128, KT * S_tot * CHUNK), dtype=bf16)
    for c in range(N_CORES):
        ids = np.where(src_of[c] >= 0, src_of[c], N_NODES)  # [128, S_tot]
        A = featbf_pad[ids]  # [128 p, S_tot, 256]
        for ci in range(N_CHUNKS):
            lo, hi = int(chunk_off[ci]), int(chunk_off[ci] + S[ci])
            blk = A[:, lo:hi, :]  # [p, Sc, 256]
            Sc = hi - lo
            # -> [i, t, s, p] -> [128, KT*Sc*128]
            b = blk.reshape(CHUNK, Sc, KT, 128).transpose(3, 2, 1, 0)
            fsrcT[c, :, KT * lo * CHUNK : KT * hi * CHUNK] = b.reshape(
                128, KT * Sc * CHUNK
            )

    meta = dict(S=S, S_tot=S_tot, chunk_off=chunk_off, perms=perms)
    in_maps = []
    for c in range(N_CORES):
        in_maps.append(
            {
                "featT": np.ascontiguousarray(featT[c]),
                "WT": WT,
                "fsrcT": np.ascontiguousarray(fsrcT[c]),
                "npad": np.ascontiguousarray(npad[c]),
                "ident": ident,
            }
        )
    return in_maps, meta


def unshard_output(results, meta):
    out = np.empty((N_NODES, HD), dtype=np.float32)
    perms = meta["perms"]
    for c in range(N_CORES):
        oc = results[c]["out"]  # [NODES_PAD, HD] rows in degree-rank order
        out[c * NODES_PER_CORE + perms[c]] = oc[:NODES_PER_CORE]
    return out


# --------------------------------------------------------------------------- #
# Bass program
# --------------------------------------------------------------------------- #
def build_program(S, S_tot, n_cores=None, nodes_pad=None):
    import concourse.bass as bass
    import concourse.bacc as bacc
    import concourse.mybir as mybir
    import concourse.tile as tile

    if n_cores is None:
        n_cores = N_CORES
    if nodes_pad is None:
        nodes_pad = NODES_PAD

    scan_op = _get_scan_op()
    f32 = mybir.dt.float32
    bf16 = mybir.dt.bfloat16
    n_chunks = len(S)
    INV_SQRT_D = 1.0 / np.sqrt(np.float32(OUT_SIZE))

    nc = bacc.Bacc(
        "TRN2",
        target_bir_lowering=False,
        debug=False,
        enable_asserts=False,
        num_devices=n_cores,
    )

    featT = nc.dram_tensor("featT", [IN_SIZE, nodes_pad], bf16, kind="ExternalInput").ap()
    WT = nc.dram_tensor("WT", [IN_SIZE, HD], bf16, kind="ExternalInput").ap()
    fsrcT = nc.dram_tensor(
        "fsrcT", [128, KT * S_tot * CHUNK], bf16, kind="ExternalInput"
    ).ap()
    npad = nc.dram_tensor("npad", [CHUNK, n_chunks], f32, kind="ExternalInput").ap()
    ident = nc.dram_tensor("ident", [CHUNK, CHUNK], bf16, kind="ExternalInput").ap()
    out = nc.dram_tensor("out", [nodes_pad, HD], f32, kind="ExternalOutput").ap()

    chunk_off = np.concatenate([[0], np.cumsum(S)])[:-1].astype(int)
    S_max = int(max(S))

    with tile.TileContext(nc) as tc:
        with (
            tc.tile_pool(name="dram", bufs=1, space="DRAM") as dram,
            tc.tile_pool(name="const", bufs=1) as cpool,
            tc.tile_pool(name="fc", bufs=3) as fcpool,
            tc.tile_pool(name="fcp", bufs=2, space="PSUM") as fcpsum,
            tc.tile_pool(name="fs", bufs=3) as fspool,
            tc.tile_pool(name="gather", bufs=3) as gpool,
            tc.tile_pool(name="scan", bufs=1) as rpool,
            tc.tile_pool(name="small", bufs=3) as spool,
            tc.tile_pool(name="pe4", bufs=2, space="PSUM") as hpsum,
            tc.tile_pool(name="psum_o", bufs=2, space="PSUM") as opsum,
        ):
            ident_sb = cpool.tile([CHUNK, CHUNK], bf16, name="ident_sb")
            nc.sync.dma_start(out=ident_sb[:], in_=ident[:])

            wt_sb = cpool.tile([128, KT * HD], bf16, name="wt_sb")
            for t in range(KT):
                nc.sync.dma_start(
                    out=wt_sb[:, t * HD : (t + 1) * HD],
                    in_=WT[t * 128 : (t + 1) * 128, :],
                )
            # ---------------- persistent edge metadata ---------------- #
            npad_sb = cpool.tile([CHUNK, n_chunks], f32, name="npad_sb")
            nc.sync.dma_start(out=npad_sb[:], in_=npad[:])

            # fp32 running-sum buffer for the score scan
            r_sb = rpool.tile([CHUNK, S_max * HD], f32, name="r_sb")

            # ---------------- main loop over chunks ---------------- #
            # The dst-side fc for chunk ci is emitted inside the loop (its
            # rows are exactly chunk ci's nodes), so the fc "prologue"
            # pipelines with the edge phase instead of serializing ahead
            # of it.
            for ci in range(n_chunks):
                Sc = int(S[ci])
                off = int(chunk_off[ci])
                SH = Sc * NUM_HEADS

                fT = fcpool.tile([128, KT * 128], bf16, tag="fT")
                for t in range(KT):
                    nc.sync.dma_start(
                        out=fT[:, t * 128 : (t + 1) * 128],
                        in_=featT[t * 128 : (t + 1) * 128, ci * 128 : (ci + 1) * 128],
                    )
                hp = fcpsum.tile([128, HD], f32, tag="hp", space="PSUM")
                for t in range(KT):
                    nc.tensor.matmul(
                        out=hp[:],
                        lhsT=fT[:, t * 128 : (t + 1) * 128],
                        rhs=wt_sb[:, t * HD : (t + 1) * HD],
                        start=(t == 0),
                        stop=(t == KT - 1),
                    )
                hdst = gpool.tile([CHUNK, HD], bf16, tag="hdst")
                nc.scalar.copy(out=hdst[:], in_=hp[:])

                # per-slot source features, PE-transposed layout
                fs_sb = fspool.tile([128, KT * S_max * CHUNK], bf16, tag="fs_sb")
                nc.sync.dma_start(
                    out=fs_sb[:, : KT * Sc * CHUNK],
                    in_=fsrcT[:, KT * off * CHUNK : KT * (off + Sc) * CHUNK],
                )

                # h_src per slot on the PE (bf16 in, fp32 PSUM), evacuated
                # to SBUF bf16 in EVAC_GROUP-slot groups, alternating the
                # Scalar and Vector engines.
                hsrc = gpool.tile([CHUNK, S_max * HD], bf16, tag="hsrc")
                for g0 in range(0, Sc, EVAC_GROUP):
                    g1 = min(g0 + EVAC_GROUP, Sc)
                    pe4 = hpsum.tile(
                        [CHUNK, EVAC_GROUP * HD], f32, tag="pe4", space="PSUM"
                    )
                    for s in range(g0, g1):
                        for t in range(KT):
                            nc.tensor.matmul(
                                out=pe4[:, (s - g0) * HD : (s - g0 + 1) * HD],
                                lhsT=fs_sb[:, (t * Sc + s) * CHUNK : (t * Sc + s + 1) * CHUNK],
                                rhs=wt_sb[:, t * HD : (t + 1) * HD],
                                start=(t == 0),
                                stop=(t == KT - 1),
                            )
                    nc.scalar.copy(
                        out=hsrc[:, g0 * HD : g1 * HD],
                        in_=pe4[:, : (g1 - g0) * HD],
                    )

                # ---- scores: r = cumsum(hsrc * hdst_bcast), fp32 state ---- #
                hdst_b = hdst[:].unsqueeze(1).broadcast_to([CHUNK, Sc, HD])
                r1 = r_sb[:, : Sc * HD]
                nc.vector._custom_dve(
                    scan_op,
                    out=r1.rearrange("p (s f) -> p s f", s=Sc),
                    in0=hsrc[:, : Sc * HD].rearrange("p (s f) -> p s f", s=Sc),
                    in1=hdst_b,
                )
                # ends at positions m*32+31, m = s*8+h (GpSimd is idle here)
                ends = spool.tile([CHUNK, S_max * NUM_HEADS + 1], f32, tag="ends")
                nc.gpsimd.memzero(ends[:, :1])
                nc.gpsimd.tensor_copy(
                    out=ends[:, 1 : SH + 1].unsqueeze(2),
                    in_=r1.rearrange("p (m d) -> p m d", d=OUT_SIZE)[:, :, 31:32],
                )
                # scoresT[p, h, s] = ends[1 + s*8 + h] - ends[s*8 + h]
                scoresT = spool.tile([CHUNK, S_max * NUM_HEADS], f32, tag="scoresT")
                nc.gpsimd.tensor_sub(
                    out=scoresT[:, :SH].rearrange("p (h s) -> p h s", h=NUM_HEADS),
                    in0=ends[:, 1 : SH + 1].rearrange("p (s h) -> p h s", h=NUM_HEADS),
                    in1=ends[:, :SH].rearrange("p (s h) -> p h s", h=NUM_HEADS),
                )
                # ex[p, h, s] = exp(scoresT / sqrt(d))  (bf16 out)
                ex = spool.tile([CHUNK, S_max * NUM_HEADS], bf16, tag="ex")
                nc.scalar.activation(
                    out=ex[:, :SH],
                    in_=scoresT[:, :SH],
                    func=mybir.ActivationFunctionType.Exp,
                    scale=float(INV_SQRT_D),
                )
                # s_t[p, h] = sum_s ex  (+npad correction, floor, reciprocal)
                s_t = spool.tile([CHUNK, NUM_HEADS], f32, tag="s_t")
                nc.vector.reduce_sum(
                    out=s_t[:].unsqueeze(2),
                    in_=ex[:, :SH].rearrange("p (h s) -> p h s", h=NUM_HEADS),
                    axis=mybir.AxisListType.X,
                )
                nc.vector.tensor_scalar(
                    out=s_t[:],
                    in0=s_t[:],
                    scalar1=npad_sb[:, ci : ci + 1],
                    scalar2=1e-30,
                    op0=mybir.AluOpType.add,
                    op1=mybir.AluOpType.max,
                )
                recip = spool.tile([CHUNK, NUM_HEADS], f32, tag="recip")
                nc.vector.reciprocal(out=recip[:], in_=s_t[:])
                # sa2[p, s, h, k] = ex[p, h, s] * recip[p, h] for k in {0, 1}
                # — the duplicated pair gives prod2's broadcast view a packed
                # innermost axis, keeping the 2x all-bf16 DVE rate without
                # materializing a full [p, s, h, d] expansion.
                sa2 = spool.tile([CHUNK, S_max * NUM_HEADS * 2], bf16, tag="sa2")
                ex_v2 = (
                    ex[:, :SH]
                    .rearrange("p (h s) -> p s h", h=NUM_HEADS)
                    .unsqueeze(1)
                    .broadcast_to([CHUNK, 2, Sc, NUM_HEADS])
                )
                recip_v2 = (
                    recip[:]
                    .unsqueeze(1)
                    .unsqueeze(2)
                    .broadcast_to([CHUNK, 2, Sc, NUM_HEADS])
                )
                nc.vector.tensor_mul(
                    out=sa2[:, : SH * 2].rearrange(
                        "p (s h two) -> p two s h", h=NUM_HEADS, two=2
                    ),
                    in0=ex_v2,
                    in1=recip_v2,
                )
                # prod2 = hsrc * sa_bcast, in place.  The first ~3/8 of the
                # slots run on the otherwise idle GpSimd (plain stride-0
                # broadcast over d), the rest on the DVE at the 2x all-bf16
                # packed rate via the duplicated-pair rank-5 view.
                half = OUT_SIZE // 2
                k_gp = (3 * Sc) // 8
                sa_shk = sa2[:, : SH * 2].rearrange(
                    "p (s h two) -> p s h two", h=NUM_HEADS, two=2
                )
                if k_gp > 0:
                    sa_gp = (
                        sa_shk[:, :k_gp, :, 0]
                        .unsqueeze(3)
                        .broadcast_to([CHUNK, k_gp, NUM_HEADS, OUT_SIZE])
                    )
                    h4_gp = hsrc[:, : k_gp * HD].rearrange(
                        "p (s h d) -> p s h d", h=NUM_HEADS, d=OUT_SIZE
                    )
                    nc.gpsimd.tensor_mul(out=h4_gp, in0=h4_gp, in1=sa_gp)
                sa_b = (
                    sa_shk[:, k_gp:Sc]
                    .unsqueeze(3)
                    .broadcast_to([CHUNK, Sc - k_gp, NUM_HEADS, half, 2])
                )
                h5 = hsrc[:, k_gp * HD : Sc * HD].rearrange(
                    "p (s h dh two) -> p s h dh two", h=NUM_HEADS, dh=half, two=2
                )
                nc.vector.tensor_mul(out=h5, in0=h5, in1=sa_b)
                # aggregation on PE: psum[p, hd] += I.T @ prod2_s
                po = opsum.tile([CHUNK, HD], f32, tag="po", space="PSUM")
                for s in range(Sc):
                    nc.tensor.matmul(
                        out=po[:],
                        lhsT=ident_sb[:],
                        rhs=hsrc[:, s * HD : (s + 1) * HD],
                        start=(s == 0),
                        stop=(s == Sc - 1),
                    )
                o_sb = spool.tile([CHUNK, HD], f32, tag="o_sb")
                nc.scalar.copy(out=o_sb[:], in_=po[:])
                nc.sync.dma_start(
                    out=out[ci * CHUNK : (ci + 1) * CHUNK, :], in_=o_sb[:]
                )

    nc.compile()
    return nc


# --------------------------------------------------------------------------- #
# Entry point
# --------------------------------------------------------------------------- #
def kernel(feat, W, src, dst, N):
    from concourse.bass_utils import run_bass_kernel_spmd

    assert int(N) == N_NODES
    in_maps, meta = build_shards(feat, W, src, dst)
    key = ("prog5", meta["S_tot"], tuple(int(x) for x in meta["S"]))
    if key in _CACHE:
        nc = _CACHE[key]
    else:
        nc = build_program(meta["S"], meta["S_tot"])
        _CACHE[key] = nc
    res = run_bass_kernel_spmd(
        nc, in_maps, core_ids=list(range(N_CORES)), trace=TRACE
    )
    globals()["LAST_RESULT"] = res
    return unshard_output(res.results, meta)


# revision 31
# speedup vs baseline: 1.5779x; 1.5779x over previous
"""DotGatConv (DGL) on 8 Trainium2 NeuronCores — v5.

Strategy (vertex-cut / dst-partitioned, host-side edge materialization):
  - Nodes are split into 8 contiguous blocks of 12500 (padded to 12544),
    degree-sorted within each core and packed 128 per chunk
    (node-per-partition); each chunk's edge slots are padded to the chunk
    max degree S_c (shared schedule across cores so the SPMD program is
    identical on every core).
  - Sharding ships, per core, the transposed source features of every edge
    slot (feat[src] pre-arranged by the host = the "all-to-all gather of
    remote source features" done at sharding time).  The device then
    computes h_src per edge slot on the PE (feat_src @ W.T, bf16 in / fp32
    PSUM), so all model FLOPs stay on device and all edge bytes stream
    through HBM as fat sequential DMA loads instead of 205k random 512B
    indirect-DMA descriptors (which bottleneck on the ~1us/instruction
    software-DGE descriptor generation).
  - Per chunk: h_src slots PE->PSUM, evacuated to SBUF bf16 (alternating
    Scalar/Vector engines); per-edge dot scores via one fused mul+running-
    sum DVE scan (fp32 state) with group sums as boundary differences
    (extraction/subtraction on the otherwise idle GpSimd); softmax with
    pad-slot correction; sa expanded over the feature dim on ACT; applied
    in-place with a 2x-rate all-bf16 DVE multiply; aggregation over edge
    slots on the PE (identity-stationary matmuls accumulating into PSUM).
  - No AllGather and no h table: the only collective-ish step is the local
    fc for the chunk's own (dst) rows.
"""

import numpy as np

IN_SIZE = 256
OUT_SIZE = 32
NUM_HEADS = 8
HD = NUM_HEADS * OUT_SIZE  # 256
N_CORES = 8
CHUNK = 128
KT = IN_SIZE // 128  # contraction k-tiles (2)

# Real-problem dimensions (overridable for scaled-down sim tests).
N_NODES = 100000
NODES_PER_CORE = N_NODES // N_CORES  # 12500
N_CHUNKS = (NODES_PER_CORE + CHUNK - 1) // CHUNK  # 98
NODES_PAD = N_CHUNKS * CHUNK  # 12544

EVAC_GROUP = 4  # slots per PSUM->SBUF evacuation copy

_CACHE = {}
TRACE = False  # set by test harness to capture an NTFF profile
LAST_RESULT = None


def _set_dims(n_nodes):
    """Recompute derived dims (used by sim tests with small graphs)."""
    global N_NODES, NODES_PER_CORE, N_CHUNKS, NODES_PAD
    N_NODES = n_nodes
    NODES_PER_CORE = N_NODES // N_CORES
    N_CHUNKS = (NODES_PER_CORE + CHUNK - 1) // CHUNK
    NODES_PAD = N_CHUNKS * CHUNK


# --------------------------------------------------------------------------- #
# Custom DVE op: out = running_sum(in0 * in1) along the free dim (fp32 state).
# --------------------------------------------------------------------------- #
def _install_custom_op():
    import concourse.dve_ops as dve_ops
    from concourse.dve_spec import Scan, Spec, Src0, Src1, AluOp, lower
    from concourse.dve_uop import DveOpSpec

    if "GAT_MUL_SCAN" in dve_ops.CUSTOM_DVE_SPECS:
        return

    def _ref_mul_scan(in0, in1, s0, s1, imm2):
        p = in0.shape[0]
        a = np.asarray(in0, np.float32).reshape(p, -1)
        b = np.asarray(in1, np.float32)
        if b.size != a.size:
            b = np.broadcast_to(b.reshape(p, -1), a.shape)
        else:
            b = b.reshape(p, -1)
        prod = a * b
        return np.cumsum(prod, axis=1, dtype=np.float32).astype(np.float32)

    spec = Spec(body=Scan(AluOp.ADD, Src0 * Src1), reference=_ref_mul_scan)
    shas = {}
    for ver in ("v3", "v4"):
        uops = lower(spec, ver=ver)
        shas[ver] = DveOpSpec(
            name="GAT_MUL_SCAN", opcode=0, uops=uops, rd1_en=True
        ).sha(ver)
    op = dve_ops.DveOp("GAT_MUL_SCAN", spec, subdim=False, uops_sha=shas)
    dve_ops.OPS.append(op)
    dve_ops.CUSTOM_DVE_SPECS[op.name] = op.spec
    dve_ops._SUB_OPCODE_FOR_NAME[op.name] = dve_ops._CUSTOM_DVE_ROW_BASE + len(dve_ops.OPS) - 1


def _get_scan_op():
    import concourse.dve_ops as dve_ops

    _install_custom_op()
    for op in dve_ops.OPS:
        if op.name == "GAT_MUL_SCAN":
            return op
    raise RuntimeError("GAT_MUL_SCAN not installed")


# --------------------------------------------------------------------------- #
# Host-side sharding: group edges by dst core / degree-sorted node chunks.
# --------------------------------------------------------------------------- #
def build_shards(feat, W, src, dst):
    import ml_dtypes

    bf16 = ml_dtypes.bfloat16
    feat = np.ascontiguousarray(np.asarray(feat, dtype=np.float32))
    W = np.ascontiguousarray(np.asarray(W, dtype=np.float32))
    src = np.asarray(src).astype(np.int64)
    dst = np.asarray(dst).astype(np.int64)
    E = src.shape[0]

    dst_core = dst // NODES_PER_CORE
    dst_local = dst - dst_core * NODES_PER_CORE

    deg = np.bincount(dst, minlength=N_NODES)  # [N]

    # Degree-sort nodes within each core; identical rank structure per core.
    perms = np.empty((N_CORES, NODES_PER_CORE), dtype=np.int64)
    degs_sorted = np.empty((N_CORES, NODES_PER_CORE), dtype=np.int64)
    for c in range(N_CORES):
        d = deg[c * NODES_PER_CORE : (c + 1) * NODES_PER_CORE]
        p = np.argsort(d, kind="stable")
        perms[c] = p
        degs_sorted[c] = d[p]

    # Shared chunk schedule: S_c = max degree among rank-slice across cores.
    S = np.zeros(N_CHUNKS, dtype=np.int64)
    for c in range(N_CHUNKS):
        lo, hi = c * CHUNK, min((c + 1) * CHUNK, NODES_PER_CORE)
        S[c] = int(degs_sorted[:, lo:hi].max()) if hi > lo else 0
    S = np.maximum(S, 1)  # avoid zero-width chunks
    S_tot = int(S.sum())
    chunk_off = np.concatenate([[0], np.cumsum(S)])[:-1]

    # rank of each node within its core (inverse permutation)
    rank_of_local = np.empty((N_CORES, NODES_PER_CORE), dtype=np.int64)
    for c in range(N_CORES):
        rank_of_local[c, perms[c]] = np.arange(NODES_PER_CORE)

    # per-edge: core, rank, slot-within-node
    e_rank = rank_of_local[dst_core, dst_local]  # [E]
    order = np.lexsort((np.arange(E), e_rank, dst_core))  # stable by (core, rank)
    sorted_key = dst_core[order] * NODES_PER_CORE + e_rank[order]
    first = np.concatenate([[True], sorted_key[1:] != sorted_key[:-1]])
    grp_start = np.where(first)[0]
    grp_id = np.cumsum(first) - 1
    slot_sorted = np.arange(E) - grp_start[grp_id]
    slot = np.empty(E, dtype=np.int64)
    slot[order] = slot_sorted

    e_chunk = e_rank // CHUNK
    e_part = e_rank % CHUNK
    col = chunk_off[e_chunk] + slot  # column in [128, S_tot]

    # src node id per (core, partition, slot column); -1 marks pad slots
    src_of = np.full((N_CORES, CHUNK, S_tot), -1, dtype=np.int64)
    src_of[dst_core, e_part, col] = src

    # -(number of pad slots) per (partition, chunk), per core.  Pad slots
    # have zero source features -> score 0 -> ex = exp(0) = 1 exactly,
    # corrected by subtracting n_pad from the softmax denominator.
    npad = np.zeros((N_CORES, CHUNK, N_CHUNKS), dtype=np.float32)
    for c in range(N_CORES):
        dd = np.zeros(NODES_PAD, dtype=np.int64)
        dd[:NODES_PER_CORE] = degs_sorted[c]
        npad[c] = -(S[None, :] - dd.reshape(N_CHUNKS, CHUNK).T).astype(np.float32)

    featbf = feat.astype(bf16)
    featbf_pad = np.concatenate(
        [featbf, np.zeros((1, IN_SIZE), dtype=bf16)], axis=0
    )  # row N_NODES = zeros for pad slots

    # featT padded per core (dst-side fc), columns in degree-rank order
    featT = np.zeros((N_CORES, IN_SIZE, NODES_PAD), dtype=bf16)
    for c in range(N_CORES):
        featT[c, :, :NODES_PER_CORE] = featbf[c * NODES_PER_CORE + perms[c]].T
    WT = np.ascontiguousarray(W.T).astype(bf16)  # [IN, HD]
    ident = np.eye(CHUNK, dtype=bf16)

    # Pre-arranged per-edge source features, transposed for the PE:
    # per chunk c a [128, KT*S_c*128] block whose column (t*S_c + s)*128 + p
    # holds feat[src(c, p, s), t*128 + i] at partition i.
    fsrcT = np.empty((N_CORES, 128, KT * S_tot * CHUNK), dtype=bf16)
    for c in range(N_CORES):
        ids = np.where(src_of[c] >= 0, src_of[c], N_NODES)  # [128, S_tot]
        A = featbf_pad[ids]  # [128 p, S_tot, 256]
        for ci in range(N_CHUNKS):
            lo, hi = int(chunk_off[ci]), int(chunk_off[ci] + S[ci])
            blk = A[:, lo:hi, :]  # [p, Sc, 256]
            Sc = hi - lo
            # -> [i, t, s, p] -> [128, KT*Sc*128]
            b = blk.reshape(CHUNK, Sc, KT, 128).transpose(3, 2, 1, 0)
            fsrcT[c, :, KT * lo * CHUNK : KT * hi * CHUNK] = b.reshape(
                128, KT * Sc * CHUNK
            )

    meta = dict(S=S, S_tot=S_tot, chunk_off=chunk_off, perms=perms)
    in_maps = []
    for c in range(N_CORES):
        in_maps.append(
            {
                "featT": np.ascontiguousarray(featT[c]),
                "WT": WT,
                "fsrcT": np.ascontiguousarray(fsrcT[c]),
                "npad": np.ascontiguousarray(npad[c]),
                "ident": ident,
            }
        )
    return in_maps, meta


def unshard_output(results, meta):
    out = np.empty((N_NODES, HD), dtype=np.float32)
    perms = meta["perms"]
    for c in range(N_CORES):
        oc = results[c]["out"]  # [NODES_PAD, HD] rows in degree-rank order
        out[c * NODES_PER_CORE + perms[c]] = oc[:NODES_PER_CORE]
    return out


# --------------------------------------------------------------------------- #
# Bass program
# --------------------------------------------------------------------------- #
def build_program(S, S_tot, n_cores=None, nodes_pad=None):
    import concourse.bass as bass
    import concourse.bacc as bacc
    import concourse.mybir as mybir
    import concourse.tile as tile

    if n_cores is None:
        n_cores = N_CORES
    if nodes_pad is None:
        nodes_pad = NODES_PAD

    scan_op = _get_scan_op()
    f32 = mybir.dt.float32
    bf16 = mybir.dt.bfloat16
    n_chunks = len(S)
    INV_SQRT_D = 1.0 / np.sqrt(np.float32(OUT_SIZE))

    nc = bacc.Bacc(
        "TRN2",
        target_bir_lowering=False,
        debug=False,
        enable_asserts=False,
        num_devices=n_cores,
    )

    featT = nc.dram_tensor("featT", [IN_SIZE, nodes_pad], bf16, kind="ExternalInput").ap()
    WT = nc.dram_tensor("WT", [IN_SIZE, HD], bf16, kind="ExternalInput").ap()
    fsrcT = nc.dram_tensor(
        "fsrcT", [128, KT * S_tot * CHUNK], bf16, kind="ExternalInput"
    ).ap()
    npad = nc.dram_tensor("npad", [CHUNK, n_chunks], f32, kind="ExternalInput").ap()
    ident = nc.dram_tensor("ident", [CHUNK, CHUNK], bf16, kind="ExternalInput").ap()
    out = nc.dram_tensor("out", [nodes_pad, HD], f32, kind="ExternalOutput").ap()

    chunk_off = np.concatenate([[0], np.cumsum(S)])[:-1].astype(int)
    S_max = int(max(S))

    with tile.TileContext(nc) as tc:
        with (
            tc.tile_pool(name="dram", bufs=1, space="DRAM") as dram,
            tc.tile_pool(name="const", bufs=1) as cpool,
            tc.tile_pool(name="fc", bufs=3) as fcpool,
            tc.tile_pool(name="fcp", bufs=2, space="PSUM") as fcpsum,
            tc.tile_pool(name="fs", bufs=3) as fspool,
            tc.tile_pool(name="gather", bufs=3) as gpool,
            tc.tile_pool(name="scan", bufs=1) as rpool,
            tc.tile_pool(name="small", bufs=3) as spool,
            tc.tile_pool(name="pe4", bufs=2, space="PSUM") as hpsum,
            tc.tile_pool(name="psum_o", bufs=2, space="PSUM") as opsum,
        ):
            ident_sb = cpool.tile([CHUNK, CHUNK], bf16, name="ident_sb")
            nc.sync.dma_start(out=ident_sb[:], in_=ident[:])

            wt_sb = cpool.tile([128, KT * HD], bf16, name="wt_sb")
            for t in range(KT):
                nc.sync.dma_start(
                    out=wt_sb[:, t * HD : (t + 1) * HD],
                    in_=WT[t * 128 : (t + 1) * 128, :],
                )
            # ---------------- persistent edge metadata ---------------- #
            npad_sb = cpool.tile([CHUNK, n_chunks], f32, name="npad_sb")
            nc.sync.dma_start(out=npad_sb[:], in_=npad[:])

            # fp32 running-sum buffer for the score scan
            r_sb = rpool.tile([CHUNK, S_max * HD], f32, name="r_sb")

            # ---------------- main loop over chunks ---------------- #
            # The dst-side fc for chunk ci is emitted inside the loop (its
            # rows are exactly chunk ci's nodes), so the fc "prologue"
            # pipelines with the edge phase instead of serializing ahead
            # of it.
            for ci in range(n_chunks):
                Sc = int(S[ci])
                off = int(chunk_off[ci])
                SH = Sc * NUM_HEADS

                fT = fcpool.tile([128, KT * 128], bf16, tag="fT")
                for t in range(KT):
                    nc.sync.dma_start(
                        out=fT[:, t * 128 : (t + 1) * 128],
                        in_=featT[t * 128 : (t + 1) * 128, ci * 128 : (ci + 1) * 128],
                    )
                hp = fcpsum.tile([128, HD], f32, tag="hp", space="PSUM")
                for t in range(KT):
                    nc.tensor.matmul(
                        out=hp[:],
                        lhsT=fT[:, t * 128 : (t + 1) * 128],
                        rhs=wt_sb[:, t * HD : (t + 1) * HD],
                        start=(t == 0),
                        stop=(t == KT - 1),
                    )
                hdst = gpool.tile([CHUNK, HD], bf16, tag="hdst")
                nc.scalar.copy(out=hdst[:], in_=hp[:])

                # per-slot source features, PE-transposed layout
                fs_sb = fspool.tile([128, KT * S_max * CHUNK], bf16, tag="fs_sb")
                nc.sync.dma_start(
                    out=fs_sb[:, : KT * Sc * CHUNK],
                    in_=fsrcT[:, KT * off * CHUNK : KT * (off + Sc) * CHUNK],
                )

                # h_src per slot on the PE (bf16 in, fp32 PSUM), evacuated
                # to SBUF bf16 in EVAC_GROUP-slot groups, alternating the
                # Scalar and Vector engines.
                hsrc = gpool.tile([CHUNK, S_max * HD], bf16, tag="hsrc")
                for g0 in range(0, Sc, EVAC_GROUP):
                    g1 = min(g0 + EVAC_GROUP, Sc)
                    pe4 = hpsum.tile(
                        [CHUNK, EVAC_GROUP * HD], f32, tag="pe4", space="PSUM"
                    )
                    for s in range(g0, g1):
                        for t in range(KT):
                            nc.tensor.matmul(
                                out=pe4[:, (s - g0) * HD : (s - g0 + 1) * HD],
                                lhsT=fs_sb[:, (t * Sc + s) * CHUNK : (t * Sc + s + 1) * CHUNK],
                                rhs=wt_sb[:, t * HD : (t + 1) * HD],
                                start=(t == 0),
                                stop=(t == KT - 1),
                            )
                    nc.scalar.copy(
                        out=hsrc[:, g0 * HD : g1 * HD],
                        in_=pe4[:, : (g1 - g0) * HD],
                    )

                # ---- scores: r = cumsum(hsrc * hdst_bcast), fp32 state ---- #
                hdst_b = hdst[:].unsqueeze(1).broadcast_to([CHUNK, Sc, HD])
                r1 = r_sb[:, : Sc * HD]
                nc.vector._custom_dve(
                    scan_op,
                    out=r1.rearrange("p (s f) -> p s f", s=Sc),
                    in0=hsrc[:, : Sc * HD].rearrange("p (s f) -> p s f", s=Sc),
                    in1=hdst_b,
                )
                # ends at positions m*32+31, m = s*8+h (GpSimd is idle here)
                ends = spool.tile([CHUNK, S_max * NUM_HEADS + 1], f32, tag="ends")
                nc.gpsimd.memzero(ends[:, :1])
                nc.gpsimd.tensor_copy(
                    out=ends[:, 1 : SH + 1].unsqueeze(2),
                    in_=r1.rearrange("p (m d) -> p m d", d=OUT_SIZE)[:, :, 31:32],
                )
                # scoresT[p, h, s] = ends[1 + s*8 + h] - ends[s*8 + h]
                scoresT = spool.tile([CHUNK, S_max * NUM_HEADS], f32, tag="scoresT")
                nc.gpsimd.tensor_sub(
                    out=scoresT[:, :SH].rearrange("p (h s) -> p h s", h=NUM_HEADS),
                    in0=ends[:, 1 : SH + 1].rearrange("p (s h) -> p h s", h=NUM_HEADS),
                    in1=ends[:, :SH].rearrange("p (s h) -> p h s", h=NUM_HEADS),
                )
                # ex[p, h, s] = exp(scoresT / sqrt(d))  (bf16 out)
                ex = spool.tile([CHUNK, S_max * NUM_HEADS], bf16, tag="ex")
                nc.scalar.activation(
                    out=ex[:, :SH],
                    in_=scoresT[:, :SH],
                    func=mybir.ActivationFunctionType.Exp,
                    scale=float(INV_SQRT_D),
                )
                # s_t[p, h] = sum_s ex  (+npad correction, floor, reciprocal)
                s_t = spool.tile([CHUNK, NUM_HEADS], f32, tag="s_t")
                nc.vector.reduce_sum(
                    out=s_t[:].unsqueeze(2),
                    in_=ex[:, :SH].rearrange("p (h s) -> p h s", h=NUM_HEADS),
                    axis=mybir.AxisListType.X,
                )
                nc.vector.tensor_scalar(
                    out=s_t[:],
                    in0=s_t[:],
                    scalar1=npad_sb[:, ci : ci + 1],
                    scalar2=1e-30,
                    op0=mybir.AluOpType.add,
                    op1=mybir.AluOpType.max,
                )
                recip = spool.tile([CHUNK, NUM_HEADS], f32, tag="recip")
                nc.vector.reciprocal(out=recip[:], in_=s_t[:])
                # sa2[p, s, h, k] = ex[p, h, s] * recip[p, h] for k in {0, 1}
                # — the duplicated pair gives prod2's broadcast view a packed
                # innermost axis, keeping the 2x all-bf16 DVE rate without
                # materializing a full [p, s, h, d] expansion.
                sa2 = spool.tile([CHUNK, S_max * NUM_HEADS * 2], bf16, tag="sa2")
                ex_v2 = (
                    ex[:, :SH]
                    .rearrange("p (h s) -> p s h", h=NUM_HEADS)
                    .unsqueeze(1)
                    .broadcast_to([CHUNK, 2, Sc, NUM_HEADS])
                )
                recip_v2 = (
                    recip[:]
                    .unsqueeze(1)
                    .unsqueeze(2)
                    .broadcast_to([CHUNK, 2, Sc, NUM_HEADS])
                )
                nc.vector.tensor_mul(
                    out=sa2[:, : SH * 2].rearrange(
                        "p (s h two) -> p two s h", h=NUM_HEADS, two=2
                    ),
                    in0=ex_v2,
                    in1=recip_v2,
                )
                # prod2 = hsrc * sa_bcast, in place (all-bf16 packed -> 2x
                # DVE rate via the duplicated-pair rank-5 view)
                half = OUT_SIZE // 2
                sa_b = (
                    sa2[:, : SH * 2]
                    .rearrange("p (s h two) -> p s h two", h=NUM_HEADS, two=2)
                    .unsqueeze(3)
                    .broadcast_to([CHUNK, Sc, NUM_HEADS, half, 2])
                )
                h5 = hsrc[:, : Sc * HD].rearrange(
                    "p (s h dh two) -> p s h dh two", h=NUM_HEADS, dh=half, two=2
                )
                nc.vector.tensor_mul(out=h5, in0=h5, in1=sa_b)
                # aggregation on PE: psum[p, hd] += I.T @ prod2_s
                po = opsum.tile([CHUNK, HD], f32, tag="po", space="PSUM")
                for s in range(Sc):
                    nc.tensor.matmul(
                        out=po[:],
                        lhsT=ident_sb[:],
                        rhs=hsrc[:, s * HD : (s + 1) * HD],
                        start=(s == 0),
                        stop=(s == Sc - 1),
                    )
                o_sb = spool.tile([CHUNK, HD], f32, tag="o_sb")
                nc.scalar.copy(out=o_sb[:], in_=po[:])
                nc.sync.dma_start(
                    out=out[ci * CHUNK : (ci + 1) * CHUNK, :], in_=o_sb[:]
                )

    nc.compile()
    return nc


# --------------------------------------------------------------------------- #
# Entry point
# --------------------------------------------------------------------------- #
def kernel(feat, W, src, dst, N):
    from concourse.bass_utils import run_bass_kernel_spmd

    assert int(N) == N_NODES
    in_maps, meta = build_shards(feat, W, src, dst)
    key = ("prog5", meta["S_tot"], tuple(int(x) for x in meta["S"]))
    if key in _CACHE:
        nc = _CACHE[key]
    else:
        nc = build_program(meta["S"], meta["S_tot"])
        _CACHE[key] = nc
    res = run_bass_kernel_spmd(
        nc, in_maps, core_ids=list(range(N_CORES)), trace=TRACE
    )
    globals()["LAST_RESULT"] = res
    return unshard_output(res.results, meta)


# revision 33
# speedup vs baseline: 1.6638x; 1.0545x over previous
"""DotGatConv (DGL) on 8 Trainium2 NeuronCores — v5.

Strategy (vertex-cut / dst-partitioned, host-side edge materialization):
  - Nodes are split into 8 contiguous blocks of 12500 (padded to 12544),
    degree-sorted within each core and packed 128 per chunk
    (node-per-partition); each chunk's edge slots are padded to the chunk
    max degree S_c (shared schedule across cores so the SPMD program is
    identical on every core).
  - Sharding ships, per core, the transposed source features of every edge
    slot (feat[src] pre-arranged by the host = the "all-to-all gather of
    remote source features" done at sharding time).  The device then
    computes h_src per edge slot on the PE (feat_src @ W.T, bf16 in / fp32
    PSUM), so all model FLOPs stay on device and all edge bytes stream
    through HBM as fat sequential DMA loads instead of 205k random 512B
    indirect-DMA descriptors (which bottleneck on the ~1us/instruction
    software-DGE descriptor generation).
  - Per chunk: h_src slots PE->PSUM, evacuated to SBUF bf16 (alternating
    Scalar/Vector engines); per-edge dot scores via one fused mul+running-
    sum DVE scan (fp32 state) with group sums as boundary differences
    (extraction/subtraction on the otherwise idle GpSimd); softmax with
    pad-slot correction; sa expanded over the feature dim on ACT; applied
    in-place with a 2x-rate all-bf16 DVE multiply; aggregation over edge
    slots on the PE (identity-stationary matmuls accumulating into PSUM).
  - No AllGather and no h table: the only collective-ish step is the local
    fc for the chunk's own (dst) rows.
"""

import numpy as np

IN_SIZE = 256
OUT_SIZE = 32
NUM_HEADS = 8
HD = NUM_HEADS * OUT_SIZE  # 256
N_CORES = 8
CHUNK = 128
KT = IN_SIZE // 128  # contraction k-tiles (2)

# Real-problem dimensions (overridable for scaled-down sim tests).
N_NODES = 100000
NODES_PER_CORE = N_NODES // N_CORES  # 12500
N_CHUNKS = (NODES_PER_CORE + CHUNK - 1) // CHUNK  # 98
NODES_PAD = N_CHUNKS * CHUNK  # 12544

EVAC_GROUP = 4  # slots per PSUM->SBUF evacuation copy

_CACHE = {}
TRACE = False  # set by test harness to capture an NTFF profile
LAST_RESULT = None


def _set_dims(n_nodes):
    """Recompute derived dims (used by sim tests with small graphs)."""
    global N_NODES, NODES_PER_CORE, N_CHUNKS, NODES_PAD
    N_NODES = n_nodes
    NODES_PER_CORE = N_NODES // N_CORES
    N_CHUNKS = (NODES_PER_CORE + CHUNK - 1) // CHUNK
    NODES_PAD = N_CHUNKS * CHUNK


# --------------------------------------------------------------------------- #
# Custom DVE op: out = running_sum(in0 * in1) along the free dim (fp32 state).
# --------------------------------------------------------------------------- #
def _install_custom_op():
    import concourse.dve_ops as dve_ops
    from concourse.dve_spec import Scan, Spec, Src0, Src1, AluOp, lower
    from concourse.dve_uop import DveOpSpec

    if "GAT_MUL_SCAN" in dve_ops.CUSTOM_DVE_SPECS:
        return

    def _ref_mul_scan(in0, in1, s0, s1, imm2):
        p = in0.shape[0]
        a = np.asarray(in0, np.float32).reshape(p, -1)
        b = np.asarray(in1, np.float32)
        if b.size != a.size:
            b = np.broadcast_to(b.reshape(p, -1), a.shape)
        else:
            b = b.reshape(p, -1)
        prod = a * b
        return np.cumsum(prod, axis=1, dtype=np.float32).astype(np.float32)

    spec = Spec(body=Scan(AluOp.ADD, Src0 * Src1), reference=_ref_mul_scan)
    shas = {}
    for ver in ("v3", "v4"):
        uops = lower(spec, ver=ver)
        shas[ver] = DveOpSpec(
            name="GAT_MUL_SCAN", opcode=0, uops=uops, rd1_en=True
        ).sha(ver)
    op = dve_ops.DveOp("GAT_MUL_SCAN", spec, subdim=False, uops_sha=shas)
    dve_ops.OPS.append(op)
    dve_ops.CUSTOM_DVE_SPECS[op.name] = op.spec
    dve_ops._SUB_OPCODE_FOR_NAME[op.name] = dve_ops._CUSTOM_DVE_ROW_BASE + len(dve_ops.OPS) - 1


def _get_scan_op():
    import concourse.dve_ops as dve_ops

    _install_custom_op()
    for op in dve_ops.OPS:
        if op.name == "GAT_MUL_SCAN":
            return op
    raise RuntimeError("GAT_MUL_SCAN not installed")


# --------------------------------------------------------------------------- #
# Host-side sharding: group edges by dst core / degree-sorted node chunks.
# --------------------------------------------------------------------------- #
def build_shards(feat, W, src, dst):
    import ml_dtypes

    bf16 = ml_dtypes.bfloat16
    feat = np.ascontiguousarray(np.asarray(feat, dtype=np.float32))
    W = np.ascontiguousarray(np.asarray(W, dtype=np.float32))
    src = np.asarray(src).astype(np.int64)
    dst = np.asarray(dst).astype(np.int64)
    E = src.shape[0]

    dst_core = dst // NODES_PER_CORE
    dst_local = dst - dst_core * NODES_PER_CORE

    deg = np.bincount(dst, minlength=N_NODES)  # [N]

    # Degree-sort nodes within each core; identical rank structure per core.
    perms = np.empty((N_CORES, NODES_PER_CORE), dtype=np.int64)
    degs_sorted = np.empty((N_CORES, NODES_PER_CORE), dtype=np.int64)
    for c in range(N_CORES):
        d = deg[c * NODES_PER_CORE : (c + 1) * NODES_PER_CORE]
        p = np.argsort(d, kind="stable")
        perms[c] = p
        degs_sorted[c] = d[p]

    # Shared chunk schedule: S_c = max degree among rank-slice across cores.
    S = np.zeros(N_CHUNKS, dtype=np.int64)
    for c in range(N_CHUNKS):
        lo, hi = c * CHUNK, min((c + 1) * CHUNK, NODES_PER_CORE)
        S[c] = int(degs_sorted[:, lo:hi].max()) if hi > lo else 0
    S = np.maximum(S, 1)  # avoid zero-width chunks
    S_tot = int(S.sum())
    chunk_off = np.concatenate([[0], np.cumsum(S)])[:-1]

    # rank of each node within its core (inverse permutation)
    rank_of_local = np.empty((N_CORES, NODES_PER_CORE), dtype=np.int64)
    for c in range(N_CORES):
        rank_of_local[c, perms[c]] = np.arange(NODES_PER_CORE)

    # per-edge: core, rank, slot-within-node
    e_rank = rank_of_local[dst_core, dst_local]  # [E]
    order = np.lexsort((np.arange(E), e_rank, dst_core))  # stable by (core, rank)
    sorted_key = dst_core[order] * NODES_PER_CORE + e_rank[order]
    first = np.concatenate([[True], sorted_key[1:] != sorted_key[:-1]])
    grp_start = np.where(first)[0]
    grp_id = np.cumsum(first) - 1
    slot_sorted = np.arange(E) - grp_start[grp_id]
    slot = np.empty(E, dtype=np.int64)
    slot[order] = slot_sorted

    e_chunk = e_rank // CHUNK
    e_part = e_rank % CHUNK
    col = chunk_off[e_chunk] + slot  # column in [128, S_tot]

    # src node id per (core, partition, slot column); -1 marks pad slots
    src_of = np.full((N_CORES, CHUNK, S_tot), -1, dtype=np.int64)
    src_of[dst_core, e_part, col] = src

    # -(number of pad slots) per (partition, chunk), per core.  Pad slots
    # have zero source features -> score 0 -> ex = exp(0) = 1 exactly,
    # corrected by subtracting n_pad from the softmax denominator.
    npad = np.zeros((N_CORES, CHUNK, N_CHUNKS), dtype=np.float32)
    for c in range(N_CORES):
        dd = np.zeros(NODES_PAD, dtype=np.int64)
        dd[:NODES_PER_CORE] = degs_sorted[c]
        npad[c] = -(S[None, :] - dd.reshape(N_CHUNKS, CHUNK).T).astype(np.float32)

    featbf = feat.astype(bf16)
    featbf_pad = np.concatenate(
        [featbf, np.zeros((1, IN_SIZE), dtype=bf16)], axis=0
    )  # row N_NODES = zeros for pad slots

    # featT padded per core (dst-side fc), columns in degree-rank order
    featT = np.zeros((N_CORES, IN_SIZE, NODES_PAD), dtype=bf16)
    for c in range(N_CORES):
        featT[c, :, :NODES_PER_CORE] = featbf[c * NODES_PER_CORE + perms[c]].T
    WT = np.ascontiguousarray(W.T).astype(bf16)  # [IN, HD]
    ident = np.eye(CHUNK, dtype=bf16)

    # Pre-arranged per-edge source features, transposed for the PE:
    # per chunk c a [128, KT*S_c*128] block whose column (t*S_c + s)*128 + p
    # holds feat[src(c, p, s), t*128 + i] at partition i.
    fsrcT = np.empty((N_CORES, 128, KT * S_tot * CHUNK), dtype=bf16)
    for c in range(N_CORES):
        ids = np.where(src_of[c] >= 0, src_of[c], N_NODES)  # [128, S_tot]
        A = featbf_pad[ids]  # [128 p, S_tot, 256]
        for ci in range(N_CHUNKS):
            lo, hi = int(chunk_off[ci]), int(chunk_off[ci] + S[ci])
            blk = A[:, lo:hi, :]  # [p, Sc, 256]
            Sc = hi - lo
            # -> [i, t, s, p] -> [128, KT*Sc*128]
            b = blk.reshape(CHUNK, Sc, KT, 128).transpose(3, 2, 1, 0)
            fsrcT[c, :, KT * lo * CHUNK : KT * hi * CHUNK] = b.reshape(
                128, KT * Sc * CHUNK
            )

    meta = dict(S=S, S_tot=S_tot, chunk_off=chunk_off, perms=perms)
    in_maps = []
    for c in range(N_CORES):
        in_maps.append(
            {
                "featT": np.ascontiguousarray(featT[c]),
                "WT": WT,
                "fsrcT": np.ascontiguousarray(fsrcT[c]),
                "npad": np.ascontiguousarray(npad[c]),
                "ident": ident,
            }
        )
    return in_maps, meta


def unshard_output(results, meta):
    out = np.empty((N_NODES, HD), dtype=np.float32)
    perms = meta["perms"]
    for c in range(N_CORES):
        oc = results[c]["out"]  # [NODES_PAD, HD] rows in degree-rank order
        out[c * NODES_PER_CORE + perms[c]] = oc[:NODES_PER_CORE]
    return out


# --------------------------------------------------------------------------- #
# Bass program
# --------------------------------------------------------------------------- #
def build_program(S, S_tot, n_cores=None, nodes_pad=None):
    import concourse.bass as bass
    import concourse.bacc as bacc
    import concourse.mybir as mybir
    import concourse.tile as tile

    if n_cores is None:
        n_cores = N_CORES
    if nodes_pad is None:
        nodes_pad = NODES_PAD

    scan_op = _get_scan_op()
    f32 = mybir.dt.float32
    bf16 = mybir.dt.bfloat16
    n_chunks = len(S)
    INV_SQRT_D = 1.0 / np.sqrt(np.float32(OUT_SIZE))

    nc = bacc.Bacc(
        "TRN2",
        target_bir_lowering=False,
        debug=False,
        enable_asserts=False,
        num_devices=n_cores,
    )

    featT = nc.dram_tensor("featT", [IN_SIZE, nodes_pad], bf16, kind="ExternalInput").ap()
    WT = nc.dram_tensor("WT", [IN_SIZE, HD], bf16, kind="ExternalInput").ap()
    fsrcT = nc.dram_tensor(
        "fsrcT", [128, KT * S_tot * CHUNK], bf16, kind="ExternalInput"
    ).ap()
    npad = nc.dram_tensor("npad", [CHUNK, n_chunks], f32, kind="ExternalInput").ap()
    ident = nc.dram_tensor("ident", [CHUNK, CHUNK], bf16, kind="ExternalInput").ap()
    out = nc.dram_tensor("out", [nodes_pad, HD], f32, kind="ExternalOutput").ap()

    chunk_off = np.concatenate([[0], np.cumsum(S)])[:-1].astype(int)
    S_max = int(max(S))

    with tile.TileContext(nc) as tc:
        with (
            tc.tile_pool(name="dram", bufs=1, space="DRAM") as dram,
            tc.tile_pool(name="const", bufs=1) as cpool,
            tc.tile_pool(name="fc", bufs=3) as fcpool,
            tc.tile_pool(name="fcp", bufs=2, space="PSUM") as fcpsum,
            tc.tile_pool(name="fs", bufs=3) as fspool,
            tc.tile_pool(name="gather", bufs=3) as gpool,
            tc.tile_pool(name="scan", bufs=1) as rpool,
            tc.tile_pool(name="small", bufs=4) as spool,
            tc.tile_pool(name="pe4", bufs=2, space="PSUM") as hpsum,
            tc.tile_pool(name="psum_o", bufs=2, space="PSUM") as opsum,
        ):
            ident_sb = cpool.tile([CHUNK, CHUNK], bf16, name="ident_sb")
            nc.sync.dma_start(out=ident_sb[:], in_=ident[:])

            wt_sb = cpool.tile([128, KT * HD], bf16, name="wt_sb")
            for t in range(KT):
                nc.sync.dma_start(
                    out=wt_sb[:, t * HD : (t + 1) * HD],
                    in_=WT[t * 128 : (t + 1) * 128, :],
                )
            # ---------------- persistent edge metadata ---------------- #
            npad_sb = cpool.tile([CHUNK, n_chunks], f32, name="npad_sb")
            nc.sync.dma_start(out=npad_sb[:], in_=npad[:])

            # fp32 running-sum buffer for the score scan
            r_sb = rpool.tile([CHUNK, S_max * HD], f32, name="r_sb")

            # ---------------- main loop over chunks ---------------- #
            # The dst-side fc for chunk ci is emitted inside the loop (its
            # rows are exactly chunk ci's nodes), so the fc "prologue"
            # pipelines with the edge phase instead of serializing ahead
            # of it.
            for ci in range(n_chunks):
                Sc = int(S[ci])
                off = int(chunk_off[ci])
                SH = Sc * NUM_HEADS

                fT = fcpool.tile([128, KT * 128], bf16, tag="fT")
                for t in range(KT):
                    nc.sync.dma_start(
                        out=fT[:, t * 128 : (t + 1) * 128],
                        in_=featT[t * 128 : (t + 1) * 128, ci * 128 : (ci + 1) * 128],
                    )
                hp = fcpsum.tile([128, HD], f32, tag="hp", space="PSUM")
                for t in range(KT):
                    nc.tensor.matmul(
                        out=hp[:],
                        lhsT=fT[:, t * 128 : (t + 1) * 128],
                        rhs=wt_sb[:, t * HD : (t + 1) * HD],
                        start=(t == 0),
                        stop=(t == KT - 1),
                    )
                hdst = gpool.tile([CHUNK, HD], bf16, tag="hdst")
                nc.scalar.copy(out=hdst[:], in_=hp[:])

                # per-slot source features, PE-transposed layout
                fs_sb = fspool.tile([128, KT * S_max * CHUNK], bf16, tag="fs_sb")
                nc.sync.dma_start(
                    out=fs_sb[:, : KT * Sc * CHUNK],
                    in_=fsrcT[:, KT * off * CHUNK : KT * (off + Sc) * CHUNK],
                )

                # h_src per slot on the PE (bf16 in, fp32 PSUM), evacuated
                # to SBUF bf16 in EVAC_GROUP-slot groups, alternating the
                # Scalar and Vector engines.
                hsrc = gpool.tile([CHUNK, S_max * HD], bf16, tag="hsrc")
                for g0 in range(0, Sc, EVAC_GROUP):
                    g1 = min(g0 + EVAC_GROUP, Sc)
                    pe4 = hpsum.tile(
                        [CHUNK, EVAC_GROUP * HD], f32, tag="pe4", space="PSUM"
                    )
                    for s in range(g0, g1):
                        for t in range(KT):
                            nc.tensor.matmul(
                                out=pe4[:, (s - g0) * HD : (s - g0 + 1) * HD],
                                lhsT=fs_sb[:, (t * Sc + s) * CHUNK : (t * Sc + s + 1) * CHUNK],
                                rhs=wt_sb[:, t * HD : (t + 1) * HD],
                                start=(t == 0),
                                stop=(t == KT - 1),
                            )
                    nc.scalar.copy(
                        out=hsrc[:, g0 * HD : g1 * HD],
                        in_=pe4[:, : (g1 - g0) * HD],
                    )

                # ---- scores: r = cumsum(hsrc * hdst_bcast), fp32 state ---- #
                hdst_b = hdst[:].unsqueeze(1).broadcast_to([CHUNK, Sc, HD])
                r1 = r_sb[:, : Sc * HD]
                nc.vector._custom_dve(
                    scan_op,
                    out=r1.rearrange("p (s f) -> p s f", s=Sc),
                    in0=hsrc[:, : Sc * HD].rearrange("p (s f) -> p s f", s=Sc),
                    in1=hdst_b,
                )
                # ends at positions m*32+31, m = s*8+h (GpSimd is idle here)
                ends = spool.tile([CHUNK, S_max * NUM_HEADS + 1], f32, tag="ends")
                nc.gpsimd.memzero(ends[:, :1])
                nc.gpsimd.tensor_copy(
                    out=ends[:, 1 : SH + 1].unsqueeze(2),
                    in_=r1.rearrange("p (m d) -> p m d", d=OUT_SIZE)[:, :, 31:32],
                )
                # scoresT[p, h, s] = ends[1 + s*8 + h] - ends[s*8 + h]
                scoresT = spool.tile([CHUNK, S_max * NUM_HEADS], f32, tag="scoresT")
                nc.gpsimd.tensor_sub(
                    out=scoresT[:, :SH].rearrange("p (h s) -> p h s", h=NUM_HEADS),
                    in0=ends[:, 1 : SH + 1].rearrange("p (s h) -> p h s", h=NUM_HEADS),
                    in1=ends[:, :SH].rearrange("p (s h) -> p h s", h=NUM_HEADS),
                )
                # ex[p, h, s] = exp(scoresT / sqrt(d))  (bf16 out)
                ex = spool.tile([CHUNK, S_max * NUM_HEADS], bf16, tag="ex")
                nc.scalar.activation(
                    out=ex[:, :SH],
                    in_=scoresT[:, :SH],
                    func=mybir.ActivationFunctionType.Exp,
                    scale=float(INV_SQRT_D),
                )
                # s_t[p, h] = sum_s ex  (+npad correction, floor, reciprocal)
                s_t = spool.tile([CHUNK, NUM_HEADS], f32, tag="s_t")
                nc.vector.reduce_sum(
                    out=s_t[:].unsqueeze(2),
                    in_=ex[:, :SH].rearrange("p (h s) -> p h s", h=NUM_HEADS),
                    axis=mybir.AxisListType.X,
                )
                nc.vector.tensor_scalar(
                    out=s_t[:],
                    in0=s_t[:],
                    scalar1=npad_sb[:, ci : ci + 1],
                    scalar2=1e-30,
                    op0=mybir.AluOpType.add,
                    op1=mybir.AluOpType.max,
                )
                recip = spool.tile([CHUNK, NUM_HEADS], f32, tag="recip")
                nc.vector.reciprocal(out=recip[:], in_=s_t[:])
                # sa2[p, s, h, k] = ex[p, h, s] * recip[p, h] for k in {0, 1}
                # — the duplicated pair gives prod2's broadcast view a packed
                # innermost axis, keeping the 2x all-bf16 DVE rate without
                # materializing a full [p, s, h, d] expansion.
                sa2 = spool.tile([CHUNK, S_max * NUM_HEADS * 2], bf16, tag="sa2")
                recip_v = recip[:].unsqueeze(1).broadcast_to([CHUNK, Sc, NUM_HEADS])
                ex_v = ex[:, :SH].rearrange("p (h s) -> p s h", h=NUM_HEADS)
                sa2_v = sa2[:, : SH * 2].rearrange(
                    "p (s h two) -> p two s h", h=NUM_HEADS, two=2
                )
                for k in range(2):
                    nc.vector.tensor_mul(
                        out=sa2_v[:, k], in0=ex_v, in1=recip_v
                    )
                # prod2 = hsrc * sa_bcast, in place (all-bf16 packed -> 2x
                # DVE rate via the duplicated-pair rank-5 view)
                half = OUT_SIZE // 2
                sa_b = (
                    sa2[:, : SH * 2]
                    .rearrange("p (s h two) -> p s h two", h=NUM_HEADS, two=2)
                    .unsqueeze(3)
                    .broadcast_to([CHUNK, Sc, NUM_HEADS, half, 2])
                )
                h5 = hsrc[:, : Sc * HD].rearrange(
                    "p (s h dh two) -> p s h dh two", h=NUM_HEADS, dh=half, two=2
                )
                nc.vector.tensor_mul(out=h5, in0=h5, in1=sa_b)
                # aggregation on PE: psum[p, hd] += I.T @ prod2_s
                po = opsum.tile([CHUNK, HD], f32, tag="po", space="PSUM")
                for s in range(Sc):
                    nc.tensor.matmul(
                        out=po[:],
                        lhsT=ident_sb[:],
                        rhs=hsrc[:, s * HD : (s + 1) * HD],
                        start=(s == 0),
                        stop=(s == Sc - 1),
                    )
                o_sb = spool.tile([CHUNK, HD], f32, tag="o_sb")
                nc.scalar.copy(out=o_sb[:], in_=po[:])
                nc.sync.dma_start(
                    out=out[ci * CHUNK : (ci + 1) * CHUNK, :], in_=o_sb[:]
                )

    nc.compile()
    return nc


# --------------------------------------------------------------------------- #
# Entry point
# --------------------------------------------------------------------------- #
def kernel(feat, W, src, dst, N):
    from concourse.bass_utils import run_bass_kernel_spmd

    assert int(N) == N_NODES
    in_maps, meta = build_shards(feat, W, src, dst)
    key = ("prog5", meta["S_tot"], tuple(int(x) for x in meta["S"]))
    if key in _CACHE:
        nc = _CACHE[key]
    else:
        nc = build_program(meta["S"], meta["S_tot"])
        _CACHE[key] = nc
    res = run_bass_kernel_spmd(
        nc, in_maps, core_ids=list(range(N_CORES)), trace=TRACE
    )
    globals()["LAST_RESULT"] = res
    return unshard_output(res.results, meta)
